# revision 4
# baseline (speedup 1.0000x reference)
"""Trainium2 Bass kernel for nn_DiscreteAutoregressiveFlow (sampling, forward).

Math: `inputs` is an exact one-hot [B, L, V] tensor. For a row holding token v:
  net = W[v] + b                      (exact: one-hot @ W picks a row)
  loc = one_hot(argmax(net[:V]));  scale = one_hot(argmax(net[V:]))
  one_hot_multiply -> one-hot at (scale_tok*v) % V   (zero row if scale_tok==0)
  one_hot_add      -> one-hot at (scale_tok*v + loc_tok) % V
So out[row] = one_hot(cmap[v]) with a host-precomputed 64-entry map
(sentinel >= V encodes the zero row). The straight-through softmax residuals
and FFT noise in the reference are O(1e-7) and vanish in norm relative error.

Device pipeline per 128x(R*64) chunk (pure streaming, memory-bound):
  xt   = DMA-in (sync HWDGE ring; all 8 issued up front, FIFO drain)
  prod = xt + cmap/128                (DVE TT add, broadcast cmap row)
  m    = reduce_max(prod, inner V)    (DVE) = 1 + cmap[tok]/128, exact
  out  = is_equal(1 + iota/128, m)    (GpSimd TT, double-broadcast APs)
  DMA-out (scalar/ACT HWDGE ring)
All f32 values involved are exact (c <= 127 with 2^-7 scaling), so the
comparison is exact. Engine split: DVE carries add+max (~2.3us/chunk),
GpSimd carries the one-hot compare (~1.6us/chunk), ACT only issues output
DMAs. Input and output DMAs ride different HWDGE rings so neither FIFO
blocks the other (the old version had all DMAs on the sync ring, which
serialized chunk c's input behind chunk c-1's compute).
Sharding: pure data parallel over B*L rows, 8 cores, no collectives.
"""

import numpy as np

V = 64
P = 128
N_CORES = 8
B, L = 16, 8192
ROWS = B * L                      # 131072
ROWS_PER_CORE = ROWS // N_CORES   # 16384
SENTINEL = 100.0
EPS = 1.0 / 128.0

# rows per partition per chunk; chunk = [128, R*64] f32 = R*32KB
R = 16

_CACHE = {}


def _build_nc(rows_per_core: int, r: int):
    import concourse.bacc as bacc
    import concourse.mybir as mybir
    from concourse.bass import broadcast_tensor_aps
    from concourse.tile import TileContext

    f32 = mybir.dt.float32
    fd = r * V
    chunk_rows = P * r
    n_chunks = rows_per_core // chunk_rows
    assert rows_per_core % chunk_rows == 0

    # Bacc (not raw Bass): its compile() runs generate_event_semaphores(),
    # which legalizes multi-wait instructions for TRN2 (1 wait per instr).
    nc = bacc.Bacc("TRN2", target_bir_lowering=False, name="daf_onehot")
    x = nc.dram_tensor("x", [rows_per_core, V], f32, kind="ExternalInput")
    cmap = nc.dram_tensor("cmap", [P, V], f32, kind="ExternalInput")
    iota = nc.dram_tensor("iota", [P, V], f32, kind="ExternalInput")
    y = nc.dram_tensor("y", [rows_per_core, V], f32, kind="ExternalOutput")

    xv = x.rearrange("(c p r) v -> c p (r v)", p=P, r=r)
    yv = y.rearrange("(c p r) v -> c p (r v)", p=P, r=r)

    with TileContext(nc) as tc:
        with (
            tc.tile_pool(name="const", bufs=1) as constp,
            tc.tile_pool(name="io", bufs=n_chunks) as iop,
            tc.tile_pool(name="work", bufs=n_chunks) as workp,
        ):
            cmap_st = constp.tile([P, V], f32, tag="cmap_st")
            iota_st = constp.tile([P, V], f32, tag="iota_st")
            nc.sync.dma_start(cmap_st[:], cmap[:])
            nc.sync.dma_start(iota_st[:], iota[:])
            cmap_1 = cmap_st[:].rearrange("p (o v) -> p o v", o=1)
            iota_1 = iota_st[:].rearrange("p (o v) -> p o v", o=1)

            # Hoist every input DMA: they have no dependencies (each chunk
            # owns its buffer), so the sync ring drains them back-to-back at
            # line rate instead of queueing behind output DMA waits.
            xts = []
            for ci in range(n_chunks):
                xt = iop.tile([P, fd], f32, tag="x")
                nc.sync.dma_start(xt[:], xv[ci])
                xts.append(xt)

            for ci in range(n_chunks):
                xt = xts[ci]
                x3 = xt[:].rearrange("p (r v) -> p r v", v=V)

                # The add runs on GpSimd (Pool supports TT add at the ISA
                # level; is_equal/reduce are DVE-only), freeing the DVE to
                # carry only max + is_equal (~2.26us/chunk < DMA cadence).
                prod = workp.tile([P, fd], f32, tag="prod")
                p3 = prod[:].rearrange("p (r v) -> p r v", v=V)
                cm_b, _ = broadcast_tensor_aps(cmap_1, p3)
                nc.gpsimd.tensor_tensor(p3, x3, cm_b, op=mybir.AluOpType.add)

                c_t = workp.tile([P, r], f32, tag="c")
                nc.vector.tensor_reduce(
                    c_t[:], p3, axis=mybir.AxisListType.X, op=mybir.AluOpType.max
                )

                out_t = iop.tile([P, fd], f32, tag="out")
                o3 = out_t[:].rearrange("p (r v) -> p r v", v=V)
                c3 = c_t[:].rearrange("p (r one) -> p r one", one=1)
                c3_b, _ = broadcast_tensor_aps(c3, o3)
                io_b, _ = broadcast_tensor_aps(iota_1, o3)
                nc.vector.tensor_tensor(o3, io_b, c3_b, op=mybir.AluOpType.is_equal)

                nc.scalar.dma_start(yv[ci], out_t[:])

    # Bacc.finalize runs compile(): wait-splitting (generate_event_semaphores),
    # register allocation, nop fusion. run_bass_via_pjrt serializes nc.m as-is,
    # so this must happen here.
    nc.finalize()
    return nc


def _get_nc(rows_per_core=ROWS_PER_CORE, r=R):
    key = (rows_per_core, r)
    if key not in _CACHE:
        _CACHE[key] = _build_nc(rows_per_core, r)
    return _CACHE[key]


def _host_cmap(W: np.ndarray, b: np.ndarray) -> np.ndarray:
    """64-entry map token -> output one-hot index (or sentinel for zero row)."""
    net = W.astype(np.float32) + b.astype(np.float32)[None, :]   # [V, 2V]
    loc_tok = np.argmax(net[:, :V], axis=1)                      # [V]
    scale_tok = np.argmax(net[:, V:], axis=1)                    # [V]
    t = (scale_tok * np.arange(V, dtype=np.int64) + loc_tok) % V
    return np.where(scale_tok == 0, SENTINEL, t.astype(np.float64)).astype(
        np.float32
    )


def _host_tables(W: np.ndarray, b: np.ndarray):
    cmap_eps = _host_cmap(W, b) * np.float32(EPS)                  # exact f32
    iota_eps = 1.0 + np.arange(V, dtype=np.float32) * np.float32(EPS)
    cmap_t = np.tile(cmap_eps.astype(np.float32)[None, :], (P, 1))
    iota_t = np.tile(iota_eps.astype(np.float32)[None, :], (P, 1))
    return cmap_t, iota_t


def _in_maps(inputs: np.ndarray, W: np.ndarray, b: np.ndarray):
    x = np.ascontiguousarray(inputs.astype(np.float32, copy=False).reshape(ROWS, V))
    cmap_t, iota_t = _host_tables(W, b)
    return [
        {
            "x": x[c * ROWS_PER_CORE : (c + 1) * ROWS_PER_CORE],
            "cmap": cmap_t,
            "iota": iota_t,
        }
        for c in range(N_CORES)
    ]


def kernel(inputs: np.ndarray, W: np.ndarray, b: np.ndarray) -> np.ndarray:
    from concourse import bass_utils

    nc = _get_nc()
    in_maps = _in_maps(inputs, W, b)
    res = bass_utils.run_bass_kernel_spmd(nc, in_maps, core_ids=list(range(N_CORES)))
    y = np.concatenate([r["y"] for r in res.results], axis=0)
    return y.reshape(inputs.shape).astype(inputs.dtype, copy=False)


# revision 5
# speedup vs baseline: 1.1007x; 1.1007x over previous
"""Trainium2 Bass kernel for nn_DiscreteAutoregressiveFlow (sampling, forward).

Math: `inputs` is an exact one-hot [B, L, V] tensor. For a row holding token v:
  net = W[v] + b                      (exact: one-hot @ W picks a row)
  loc = one_hot(argmax(net[:V]));  scale = one_hot(argmax(net[V:]))
  one_hot_multiply -> one-hot at (scale_tok*v) % V   (zero row if scale_tok==0)
  one_hot_add      -> one-hot at (scale_tok*v + loc_tok) % V
So out[row] = one_hot(cmap[v]) with a host-precomputed 64-entry map
(sentinel >= V encodes the zero row). The straight-through softmax residuals
and FFT noise in the reference are O(1e-7) and vanish in norm relative error.

Two device pipelines (picked per problem instance on the host):

Fast path (cmap has no sentinel / zero-row tokens, true for this instance):
  xt   = DMA-in f32 (sync HWDGE; all chunk DMAs issued up front, FIFO drain)
  xb   = bf16 cast copy (ACT engine, hoisted)
  prod = xb + cmap/128               (DVE TT add, all-bf16 -> 2x mode, ~600ns)
  m    = reduce_max(prod, inner V)   (DVE, f32 accum) = 1 + cmap[tok]/128
  idx  = (m-1)*256 + (128*(j%8)+1)   (DVE tensor_scalar + tensor_tensor, tiny)
  idx16= int16 convert               (ACT copy, tiny)
  out  = gpsimd.local_scatter: zero the tile, write bf16 1.0 at idx.
         The out tile is the bf16 *cell view* of the f32 output: f32 1.0 is
         cells [0x0000, 0x3F80], so scattering bf16 1.0 into cell 2e+1 of a
         zeroed tile builds the exact f32 one-hot row bit pattern.
  DMA-out f32 (sync ring, queued after the inputs - HBM is shared anyway)
This removes the 1024-wide broadcast-compare (IS_EQ) from the DVE entirely;
DVE work is ~1.9us/chunk, well under the ~23.5us HBM stream floor.

General path (sentinel present): same structure but the one-hot rows are
built by DVE is_equal(1 + iota/128, m) against the broadcast max (exact in
f32/bf16; sentinel max never matches any iota entry -> zero row).

Sharding: pure data parallel over B*L rows, 8 cores, no collectives.
"""

import numpy as np
import ml_dtypes

V = 64
P = 128
N_CORES = 8
B, L = 16, 8192
ROWS = B * L                      # 131072
ROWS_PER_CORE = ROWS // N_CORES   # 16384
SENTINEL = 100.0
EPS = 1.0 / 128.0

# rows per partition per chunk; chunk = [128, R*64] f32 = R*32KB
R = 16

_CACHE = {}


def _build_nc(rows_per_core: int, r: int, scatter: bool):
    import concourse.bacc as bacc
    import concourse.mybir as mybir
    from concourse.bass import broadcast_tensor_aps
    from concourse.tile import TileContext

    f32 = mybir.dt.float32
    bf16 = mybir.dt.bfloat16
    i16 = mybir.dt.int16
    fd = r * V
    chunk_rows = P * r
    n_chunks = rows_per_core // chunk_rows
    assert rows_per_core % chunk_rows == 0
    assert r == 16

    # Bacc (not raw Bass): its compile() runs generate_event_semaphores(),
    # which legalizes multi-wait instructions for TRN2 (1 wait per instr).
    nc = bacc.Bacc("TRN2", target_bir_lowering=False, name="daf_onehot")
    x = nc.dram_tensor("x", [rows_per_core, V], f32, kind="ExternalInput")
    cmap = nc.dram_tensor("cmap", [P, V], f32, kind="ExternalInput")
    iota = nc.dram_tensor("iota", [P, V], f32, kind="ExternalInput")
    off = nc.dram_tensor("off", [P, r], f32, kind="ExternalInput")
    ones = nc.dram_tensor("ones", [P, r], bf16, kind="ExternalInput")
    y = nc.dram_tensor("y", [rows_per_core, V], f32, kind="ExternalOutput")

    xv = x.rearrange("(c p r) v -> c p (r v)", p=P, r=r)
    yv = y.rearrange("(c p r) v -> c p (r v)", p=P, r=r)

    with TileContext(nc) as tc:
        with (
            tc.tile_pool(name="const", bufs=1) as constp,
            tc.tile_pool(name="io", bufs=n_chunks) as iop,
            tc.tile_pool(name="work", bufs=n_chunks) as workp,
        ):
            cmap_st = constp.tile([P, V], f32, tag="cmap_st")
            nc.sync.dma_start(cmap_st[:], cmap[:])
            if scatter:
                off_t = constp.tile([P, r], f32, tag="off_t")
                ones_t = constp.tile([P, r], bf16, tag="ones_t")
                nc.sync.dma_start(off_t[:], off[:])
                nc.sync.dma_start(ones_t[:], ones[:])
            else:
                iota_st = constp.tile([P, V], f32, tag="iota_st")
                nc.sync.dma_start(iota_st[:], iota[:])
                iota_1 = iota_st[:].rearrange("p (o v) -> p o v", o=1)

            # Hoisted input DMAs: no dependencies (each chunk owns its
            # buffer), so the sync ring drains them back-to-back at line
            # rate; output DMAs queue behind them on the same ring, which
            # costs nothing - HBM bandwidth is shared either way.
            xts = []
            for ci in range(n_chunks):
                xt = iop.tile([P, fd], f32, tag="x")
                nc.sync.dma_start(xt[:], xv[ci])
                xts.append(xt)

            # cmap broadcast-materialized bf16 so the DVE add runs in 2x
            # mode (needs every operand 2-byte, stride-1).
            cmap_1 = cmap_st[:].rearrange("p (o v) -> p o v", o=1)
            cmapf = constp.tile([P, fd], bf16, tag="cmapf")
            cf3 = cmapf[:].rearrange("p (r v) -> p r v", v=V)
            cm_b, _ = broadcast_tensor_aps(cmap_1, cf3)
            nc.scalar.copy(cf3, cm_b)

            # Hoisted bf16 cast copies on the ACT engine (keeps the DVE add
            # in 2x mode without spending a DVE pass on the convert).
            xbs = []
            for ci in range(n_chunks):
                xb = workp.tile([P, fd], bf16, tag="xb")
                nc.scalar.copy(xb[:], xts[ci][:])
                xbs.append(xb)

            for ci in range(n_chunks):
                prod = workp.tile([P, fd], bf16, tag="prod")
                nc.vector.tensor_tensor(
                    prod[:], xbs[ci][:], cmapf[:], op=mybir.AluOpType.add
                )
                p3 = prod[:].rearrange("p (r v) -> p r v", v=V)

                c_t = workp.tile([P, r], f32, tag="c")
                nc.vector.tensor_reduce(
                    c_t[:], p3, axis=mybir.AxisListType.X, op=mybir.AluOpType.max
                )

                if scatter:
                    # idx cell = 2*(j*64 + cmap[tok]) + 1 relative to the
                    # half-chunk: (m-1)*256 = 2*cmap[tok]; off = 128*(j%8)+1.
                    t_t = workp.tile([P, r], f32, tag="t")
                    nc.vector.tensor_scalar(
                        t_t[:], c_t[:], 1.0, 256.0,
                        op0=mybir.AluOpType.subtract, op1=mybir.AluOpType.mult,
                    )
                    idxf = workp.tile([P, r], f32, tag="idxf")
                    nc.vector.tensor_tensor(
                        idxf[:], t_t[:], off_t[:], op=mybir.AluOpType.add
                    )
                    idx16 = workp.tile([P, r], i16, tag="idx16")
                    nc.scalar.copy(idx16[:], idxf[:])

                    out_t = iop.tile([P, 2 * fd], bf16, tag="out")
                    oh = out_t[:].rearrange("p (h e) -> h p e", h=2)
                    ih = idx16[:].rearrange("p (h j) -> h p j", h=2)
                    dh = ones_t[:].rearrange("p (h j) -> h p j", h=2)
                    for h in range(2):
                        nc.gpsimd.local_scatter(
                            oh[h], dh[h], ih[h],
                            channels=P, num_elems=fd, num_idxs=r // 2,
                        )
                    nc.sync.dma_start(yv[ci], out_t[:].bitcast(f32))
                else:
                    out_t = iop.tile([P, fd], f32, tag="out")
                    o3 = out_t[:].rearrange("p (r v) -> p r v", v=V)
                    c3 = c_t[:].rearrange("p (r one) -> p r one", one=1)
                    c3_b, _ = broadcast_tensor_aps(c3, o3)
                    io_b, _ = broadcast_tensor_aps(iota_1, o3)
                    nc.vector.tensor_tensor(
                        o3, io_b, c3_b, op=mybir.AluOpType.is_equal
                    )
                    nc.sync.dma_start(yv[ci], out_t[:])

    # Bacc.finalize runs compile(): wait-splitting (generate_event_semaphores),
    # register allocation, nop fusion. run_bass_via_pjrt serializes nc.m as-is,
    # so this must happen here.
    nc.finalize()
    return nc


def _get_nc(rows_per_core=ROWS_PER_CORE, r=R, scatter=False):
    key = (rows_per_core, r, scatter)
    if key not in _CACHE:
        _CACHE[key] = _build_nc(rows_per_core, r, scatter)
    return _CACHE[key]


def _host_cmap(W: np.ndarray, b: np.ndarray) -> np.ndarray:
    """64-entry map token -> output one-hot index (or sentinel for zero row)."""
    net = W.astype(np.float32) + b.astype(np.float32)[None, :]   # [V, 2V]
    loc_tok = np.argmax(net[:, :V], axis=1)                      # [V]
    scale_tok = np.argmax(net[:, V:], axis=1)                    # [V]
    t = (scale_tok * np.arange(V, dtype=np.int64) + loc_tok) % V
    return np.where(scale_tok == 0, SENTINEL, t.astype(np.float64)).astype(
        np.float32
    )


def _host_tables(W: np.ndarray, b: np.ndarray):
    cmap_eps = _host_cmap(W, b) * np.float32(EPS)                  # exact f32
    iota_eps = 1.0 + np.arange(V, dtype=np.float32) * np.float32(EPS)
    cmap_t = np.tile(cmap_eps.astype(np.float32)[None, :], (P, 1))
    iota_t = np.tile(iota_eps.astype(np.float32)[None, :], (P, 1))
    return cmap_t, iota_t


def _in_maps(inputs: np.ndarray, W: np.ndarray, b: np.ndarray):
    x = np.ascontiguousarray(inputs.astype(np.float32, copy=False).reshape(ROWS, V))
    cmap_t, iota_t = _host_tables(W, b)
    off = np.tile(
        (128.0 * (np.arange(R) % (R // 2)) + 1.0).astype(np.float32)[None, :],
        (P, 1),
    )
    ones = np.ones((P, R), dtype=ml_dtypes.bfloat16)
    return [
        {
            "x": x[c * ROWS_PER_CORE : (c + 1) * ROWS_PER_CORE],
            "cmap": cmap_t,
            "iota": iota_t,
            "off": off,
            "ones": ones,
        }
        for c in range(N_CORES)
    ]


def _use_scatter(W: np.ndarray, b: np.ndarray) -> bool:
    """Fast path is exact iff no token maps to the zero row (no sentinel)."""
    return not np.any(_host_cmap(W, b) >= np.float32(V))


def kernel(inputs: np.ndarray, W: np.ndarray, b: np.ndarray) -> np.ndarray:
    from concourse import bass_utils

    nc = _get_nc(scatter=_use_scatter(W, b))
    in_maps = _in_maps(inputs, W, b)
    res = bass_utils.run_bass_kernel_spmd(nc, in_maps, core_ids=list(range(N_CORES)))
    y = np.concatenate([r["y"] for r in res.results], axis=0)
    return y.reshape(inputs.shape).astype(inputs.dtype, copy=False)


# revision 6
# speedup vs baseline: 1.1242x; 1.0214x over previous
"""Trainium2 Bass kernel for nn_DiscreteAutoregressiveFlow (sampling, forward).

Math: `inputs` is an exact one-hot [B, L, V] tensor. For a row holding token v:
  net = W[v] + b                      (exact: one-hot @ W picks a row)
  loc = one_hot(argmax(net[:V]));  scale = one_hot(argmax(net[V:]))
  one_hot_multiply -> one-hot at (scale_tok*v) % V   (zero row if scale_tok==0)
  one_hot_add      -> one-hot at (scale_tok*v + loc_tok) % V
So out[row] = one_hot(cmap[v]) with a host-precomputed 64-entry map
(sentinel >= V encodes the zero row). The straight-through softmax residuals
and FFT noise in the reference are O(1e-7) and vanish in norm relative error.

Two device pipelines (picked per problem instance on the host):

Fast path (cmap has no sentinel / zero-row tokens, true for this instance):
  xt   = DMA-in f32 (sync HWDGE; all chunk DMAs issued up front, FIFO drain)
  xb   = bf16 cast copy (ACT engine, hoisted)
  prod = xb + cmap/128               (DVE TT add, all-bf16 -> 2x mode, ~600ns)
  m    = reduce_max(prod, inner V)   (DVE, f32 accum) = 1 + cmap[tok]/128
  idx  = (m-1)*256 + (128*(j%8)+1)   (DVE tensor_scalar + tensor_tensor, tiny)
  idx16= int16 convert               (ACT copy, tiny)
  out  = gpsimd.local_scatter: zero the tile, write bf16 1.0 at idx.
         The out tile is the bf16 *cell view* of the f32 output: f32 1.0 is
         cells [0x0000, 0x3F80], so scattering bf16 1.0 into cell 2e+1 of a
         zeroed tile builds the exact f32 one-hot row bit pattern.
  DMA-out f32 (sync ring, queued after the inputs - HBM is shared anyway)
This removes the 1024-wide broadcast-compare (IS_EQ) from the DVE entirely;
DVE work is ~1.9us/chunk, well under the ~23.5us HBM stream floor.

General path (sentinel present): same structure but the one-hot rows are
built by DVE is_equal(1 + iota/128, m) against the broadcast max (exact in
f32/bf16; sentinel max never matches any iota entry -> zero row).

Sharding: pure data parallel over B*L rows, 8 cores, no collectives.
"""

import numpy as np
import ml_dtypes

V = 64
P = 128
N_CORES = 8
B, L = 16, 8192
ROWS = B * L                      # 131072
ROWS_PER_CORE = ROWS // N_CORES   # 16384
SENTINEL = 100.0
EPS = 1.0 / 128.0

# rows per partition per chunk; chunk = [128, R*64] f32 = R*32KB
R = 16

_CACHE = {}


def _build_nc(rows_per_core: int, r: int, scatter: bool):
    import concourse.bacc as bacc
    import concourse.mybir as mybir
    from concourse.bass import broadcast_tensor_aps
    from concourse.tile import TileContext

    f32 = mybir.dt.float32
    bf16 = mybir.dt.bfloat16
    i16 = mybir.dt.int16
    fd = r * V
    chunk_rows = P * r
    n_chunks = rows_per_core // chunk_rows
    assert rows_per_core % chunk_rows == 0
    assert r == 16

    # Bacc (not raw Bass): its compile() runs generate_event_semaphores(),
    # which legalizes multi-wait instructions for TRN2 (1 wait per instr).
    nc = bacc.Bacc("TRN2", target_bir_lowering=False, name="daf_onehot")
    x = nc.dram_tensor("x", [rows_per_core, V], f32, kind="ExternalInput")
    cmap = nc.dram_tensor("cmap", [P, V], f32, kind="ExternalInput")
    iota = nc.dram_tensor("iota", [P, V], f32, kind="ExternalInput")
    off = nc.dram_tensor("off", [P, r], f32, kind="ExternalInput")
    ones = nc.dram_tensor("ones", [P, r], bf16, kind="ExternalInput")
    y = nc.dram_tensor("y", [rows_per_core, V], f32, kind="ExternalOutput")

    xv = x.rearrange("(c p r) v -> c p (r v)", p=P, r=r)
    yv = y.rearrange("(c p r) v -> c p (r v)", p=P, r=r)

    with TileContext(nc) as tc:
        with (
            tc.tile_pool(name="const", bufs=1) as constp,
            tc.tile_pool(name="io", bufs=n_chunks) as iop,
            tc.tile_pool(name="work", bufs=n_chunks) as workp,
        ):
            # Constants ride the scalar (ACT) HWDGE ring so the sync ring's
            # first descriptors belong to chunk 0's input (each HWDGE DMA
            # carries ~0.6us of serialized setup on its ring).
            cmap_st = constp.tile([P, V], f32, tag="cmap_st")
            iota_st = constp.tile([P, V], f32, tag="iota_st")
            nc.scalar.dma_start(cmap_st[:], cmap[:])
            nc.scalar.dma_start(iota_st[:], iota[:])
            iota_1 = iota_st[:].rearrange("p (o v) -> p o v", o=1)
            if scatter:
                off_t = constp.tile([P, r], f32, tag="off_t")
                ones_t = constp.tile([P, r], bf16, tag="ones_t")
                nc.scalar.dma_start(off_t[:], off[:])
                nc.scalar.dma_start(ones_t[:], ones[:])

            # Hoisted input DMAs: no dependencies (each chunk owns its
            # buffer), so the sync ring drains them back-to-back at line
            # rate; output DMAs queue behind them on the same ring, which
            # costs nothing - HBM bandwidth is shared either way.
            xts = []
            for ci in range(n_chunks):
                xt = iop.tile([P, fd], f32, tag="x")
                nc.sync.dma_start(xt[:], xv[ci])
                xts.append(xt)

            # cmap broadcast-materialized bf16 so the DVE add runs in 2x
            # mode (needs every operand 2-byte, stride-1).
            cmap_1 = cmap_st[:].rearrange("p (o v) -> p o v", o=1)
            cmapf = constp.tile([P, fd], bf16, tag="cmapf")
            cf3 = cmapf[:].rearrange("p (r v) -> p r v", v=V)
            cm_b, _ = broadcast_tensor_aps(cmap_1, cf3)
            nc.scalar.copy(cf3, cm_b)

            # bf16 cast copies on the ACT engine (keeps the DVE add in 2x
            # mode without spending a DVE pass on the convert). Two are
            # primed here; the rest are emitted inside the loop so the tiny
            # int16 index converts don't queue behind all eight copies in
            # the ACT FIFO.
            xbs = []

            def emit_xcopy(ci):
                xb = workp.tile([P, fd], bf16, tag="xb")
                nc.scalar.copy(xb[:], xts[ci][:])
                xbs.append(xb)

            emit_xcopy(0)
            emit_xcopy(1)

            # Chunk 0 builds its one-hot rows on the DVE (is_equal); chunks
            # 1..n-1 build theirs with gpsimd local_scatter. This balances
            # the two engines at ~15-16us each instead of serializing ~23us
            # of output-generation on either one.
            for ci in range(n_chunks):
                prod = workp.tile([P, fd], bf16, tag="prod")
                nc.vector.tensor_tensor(
                    prod[:], xbs[ci][:], cmapf[:], op=mybir.AluOpType.add
                )
                p3 = prod[:].rearrange("p (r v) -> p r v", v=V)

                c_t = workp.tile([P, r], f32, tag="c")
                nc.vector.tensor_reduce(
                    c_t[:], p3, axis=mybir.AxisListType.X, op=mybir.AluOpType.max
                )

                if scatter and ci > 0:
                    # idx cell = 2*(j*64 + cmap[tok]) + 1 relative to the
                    # half-chunk: (m-1)*256 = 2*cmap[tok]; off = 128*(j%8)+1.
                    t_t = workp.tile([P, r], f32, tag="t")
                    nc.vector.tensor_scalar(
                        t_t[:], c_t[:], 1.0, 256.0,
                        op0=mybir.AluOpType.subtract, op1=mybir.AluOpType.mult,
                    )
                    idxf = workp.tile([P, r], f32, tag="idxf")
                    nc.vector.tensor_tensor(
                        idxf[:], t_t[:], off_t[:], op=mybir.AluOpType.add
                    )
                    idx16 = workp.tile([P, r], i16, tag="idx16")
                    nc.scalar.copy(idx16[:], idxf[:])

                    out_t = iop.tile([P, 2 * fd], bf16, tag="out")
                    oh = out_t[:].rearrange("p (h e) -> h p e", h=2)
                    ih = idx16[:].rearrange("p (h j) -> h p j", h=2)
                    dh = ones_t[:].rearrange("p (h j) -> h p j", h=2)
                    for h in range(2):
                        nc.gpsimd.local_scatter(
                            oh[h], dh[h], ih[h],
                            channels=P, num_elems=fd, num_idxs=r // 2,
                        )
                    nc.sync.dma_start(yv[ci], out_t[:].bitcast(f32))
                else:
                    out_t = iop.tile([P, fd], f32, tag="out")
                    o3 = out_t[:].rearrange("p (r v) -> p r v", v=V)
                    c3 = c_t[:].rearrange("p (r one) -> p r one", one=1)
                    c3_b, _ = broadcast_tensor_aps(c3, o3)
                    io_b, _ = broadcast_tensor_aps(iota_1, o3)
                    nc.vector.tensor_tensor(
                        o3, io_b, c3_b, op=mybir.AluOpType.is_equal
                    )
                    nc.sync.dma_start(yv[ci], out_t[:])

                if ci + 2 < n_chunks:
                    emit_xcopy(ci + 2)

    # Bacc.finalize runs compile(): wait-splitting (generate_event_semaphores),
    # register allocation, nop fusion. run_bass_via_pjrt serializes nc.m as-is,
    # so this must happen here.
    nc.finalize()
    return nc


def _get_nc(rows_per_core=ROWS_PER_CORE, r=R, scatter=False):
    key = (rows_per_core, r, scatter)
    if key not in _CACHE:
        _CACHE[key] = _build_nc(rows_per_core, r, scatter)
    return _CACHE[key]


def _host_cmap(W: np.ndarray, b: np.ndarray) -> np.ndarray:
    """64-entry map token -> output one-hot index (or sentinel for zero row)."""
    net = W.astype(np.float32) + b.astype(np.float32)[None, :]   # [V, 2V]
    loc_tok = np.argmax(net[:, :V], axis=1)                      # [V]
    scale_tok = np.argmax(net[:, V:], axis=1)                    # [V]
    t = (scale_tok * np.arange(V, dtype=np.int64) + loc_tok) % V
    return np.where(scale_tok == 0, SENTINEL, t.astype(np.float64)).astype(
        np.float32
    )


def _host_tables(W: np.ndarray, b: np.ndarray):
    cmap_eps = _host_cmap(W, b) * np.float32(EPS)                  # exact f32
    iota_eps = 1.0 + np.arange(V, dtype=np.float32) * np.float32(EPS)
    cmap_t = np.tile(cmap_eps.astype(np.float32)[None, :], (P, 1))
    iota_t = np.tile(iota_eps.astype(np.float32)[None, :], (P, 1))
    return cmap_t, iota_t


def _in_maps(inputs: np.ndarray, W: np.ndarray, b: np.ndarray):
    x = np.ascontiguousarray(inputs.astype(np.float32, copy=False).reshape(ROWS, V))
    cmap_t, iota_t = _host_tables(W, b)
    off = np.tile(
        (128.0 * (np.arange(R) % (R // 2)) + 1.0).astype(np.float32)[None, :],
        (P, 1),
    )
    ones = np.ones((P, R), dtype=ml_dtypes.bfloat16)
    return [
        {
            "x": x[c * ROWS_PER_CORE : (c + 1) * ROWS_PER_CORE],
            "cmap": cmap_t,
            "iota": iota_t,
            "off": off,
            "ones": ones,
        }
        for c in range(N_CORES)
    ]


def _use_scatter(W: np.ndarray, b: np.ndarray) -> bool:
    """Fast path is exact iff no token maps to the zero row (no sentinel)."""
    return not np.any(_host_cmap(W, b) >= np.float32(V))


def kernel(inputs: np.ndarray, W: np.ndarray, b: np.ndarray) -> np.ndarray:
    from concourse import bass_utils

    nc = _get_nc(scatter=_use_scatter(W, b))
    in_maps = _in_maps(inputs, W, b)
    res = bass_utils.run_bass_kernel_spmd(nc, in_maps, core_ids=list(range(N_CORES)))
    y = np.concatenate([r["y"] for r in res.results], axis=0)
    return y.reshape(inputs.shape).astype(inputs.dtype, copy=False)
